# revision 26
# baseline (speedup 1.0000x reference)
"""MLA forward kernel for Trainium2, 8 NeuronCores.

Sharding: data-parallel over batch (2) x tensor-parallel over heads (16 -> 4
groups of 4). Core c handles batch c//4, head group c%4. kv compression is
replicated per core. Each core emits a partial [S, D] output (its heads'
contribution through out_proj) in bf16; the host sums the 4 partials per batch
in fp32.

Key design points:
  - x-side projections (kv latent, q_nope, q_rope) in fp8e4 hi/lo DoubleRow
    matmuls (3 products), weights pre-scaled by 32. x packed [128, KP, 2, 512]
    per 512-row super so q_nope uses a 512-wide moving operand and comes out
    head-major (no transpose); k_nope likewise 512 wide per (head, super).
  - one activation table for the whole kernel ({Square, Ln, Exp, Copy},
    pinned via _pin_act_tables): rmsnorm rstd = Exp(-0.5*Ln(var/R + eps)) on
    Act. This avoids ~33 LoadActFuncSet swaps (1.3us each) that bass's greedy
    per-instruction table choice would otherwise emit.
  - kv latent is kept one super deep ([128,512] ring): stage B of super j
    consumes it before stage A of super j+1 rewrites it (PE program order).
  - attention: scores^T per 128x128 block (exact causal block-triangle),
    P^T = exp(scale*s) bf16; AV fused with the softmax row-sum via a
    ones-column on V; normalization = reciprocal_approx_fast + per-partition
    tensor_scalar mul. Deep pt/fo/oq rings keep the exp->AV->out_proj
    pipeline fed.
  - rope rotations split across DVE and Pool from a bf16 SBUF copy of the
    psum tile; transposes on PE via deferred-emission queues that fill PE
    dependency stalls (rope/kv/attention-output transposes + out_proj chunks).
  - out_proj in fp8 hi/lo DoubleRow (head-pairs packed), host divides by 32.
"""

import sys
import numpy as np
import ml_dtypes

sys.path.insert(0, "/opt/trn_rl_repo")

import concourse.bass as bass  # noqa: E402
import concourse.tile as tile  # noqa: E402
from concourse import mybir, bacc  # noqa: E402
from concourse.bass_utils import run_bass_kernel_spmd  # noqa: E402
from concourse.masks import make_identity  # noqa: E402
from concourse.alu_op_type import AluOpType  # noqa: E402
from contextlib import ExitStack  # noqa: E402

B, S, D = 2, 2048, 2048
H, DN, DR, DV, R = 16, 128, 64, 128, 512
HL = 4  # heads per core
EPS = 1e-6
WS = 32.0  # fp8 weight pre-scale
SCALE = 1.0 / (float(np.sqrt(DN + DR)) * WS)
BF = mybir.dt.bfloat16
F32 = mybir.dt.float32
F8 = mybir.dt.float8e4
NT = S // 128   # 16 s-tiles
NJ = 4          # s-supers of 512
KP = 8          # D packs of 256
DRM = mybir.MatmulPerfMode.DoubleRow
AT = mybir.ActivationFunctionType

_CACHE = {}


def _pin_act_tables():
    """Route every activation to the one table set holding all four
    functions we use ({Square, Ln, Exp, Copy}); otherwise bass's greedy
    per-instruction set choice thrashes LoadActFuncSet ~33x per kernel.
    Indices are preserved (only membership is masked) so the emitted
    act_func_set_id still matches act_info.json."""
    import concourse.bacc as _bacc_mod
    if getattr(_bacc_mod, "_act_tables_pinned", False):
        return
    orig = _bacc_mod.get_activation_tables
    keep = "natural_log_exp_and_others"

    def pinned(arch):
        t = orig(arch)
        return {k: (v if k == keep else type(v)()) for k, v in t.items()}

    _bacc_mod.get_activation_tables = pinned
    _bacc_mod._act_tables_pinned = True


def _build():
    _pin_act_tables()
    nc = bacc.Bacc("TRN2", target_bir_lowering=False, debug=False)

    def din(name, shape, dt):
        return nc.dram_tensor(name, list(shape), dt, kind="ExternalInput").ap()

    xh_d = din("xh", [NJ, 128, KP, 2, 512], F8)
    xl_d = din("xl", [NJ, 128, KP, 2, 512], F8)
    wkvh_d = din("wkvh", [128, KP, 2, R], F8)
    wkvl_d = din("wkvl", [128, KP, 2, R], F8)
    wqnh_d = din("wqnh", [128, KP, 2, HL * DN], F8)
    wqnl_d = din("wqnl", [128, KP, 2, HL * DN], F8)
    wqrh_d = din("wqrh", [128, KP, 2, HL * DR], F8)
    wqrl_d = din("wqrl", [128, KP, 2, HL * DR], F8)
    wkn_d = din("wkn", [128, 4, HL * DN], BF)
    wkr_d = din("wkr", [128, 4, HL * DR], BF)
    wv_d = din("wv", [128, 4, HL * DV], BF)
    woh_d = din("woh", [2, 128, 2, D], F8)
    wol_d = din("wol", [2, 128, 2, D], F8)
    cos_d = din("cosb", [128, NT, 128], BF)
    sin_d = din("sinb", [128, NT, 128], BF)
    out_d = nc.dram_tensor("out", [S, D], BF, kind="ExternalOutput").ap()

    with tile.TileContext(nc) as tc, ExitStack() as outer:
        pp = outer.enter_context(tc.tile_pool(name="persist", bufs=1))
        ident = pp.tile([128, 128], BF, tag="ident", name="ident")
        bmask = pp.tile([128, 128], BF, tag="bmask", name="bmask")
        epst = pp.tile([128, 1], F32, tag="epst", name="epst")
        QnT = [pp.tile([128, S], BF, tag=f"QnT{h}", name=f"QnT{h}")
               for h in range(HL)]
        KnT = [pp.tile([128, S], BF, tag=f"KnT{h}", name=f"KnT{h}")
               for h in range(HL)]
        QrT = [pp.tile([128, S], BF, tag=f"QrT{r}", name=f"QrT{r}")
               for r in range(2)]
        KrT = [pp.tile([128, S], BF, tag=f"KrT{r}", name=f"KrT{r}")
               for r in range(2)]
        # kv latent, one super deep (stage B of super j consumes it before
        # stage A of super j+1 rewrites it; PE program order guarantees this)
        kvT = [pp.tile([128, 512], BF, tag=f"kvT{r}", name=f"kvT{r}")
               for r in range(4)]
        Vg = [pp.tile([128, HL, DV + 1], BF, tag=f"Vg{t}", name=f"Vg{t}")
              for t in range(NT)]
        woh_t = [pp.tile([128, 2, D], F8, tag=f"woh{p}", name=f"woh{p}")
                 for p in range(2)]
        wol_t = [pp.tile([128, 2, D], F8, tag=f"wol{p}", name=f"wol{p}")
                 for p in range(2)]
        wkn_t = pp.tile([128, 4, HL * DN], BF, tag="wkn", name="wkn")
        wkr_t = pp.tile([128, 4, HL * DR], BF, tag="wkr", name="wkr")
        wv_t = pp.tile([128, 4, HL * DV], BF, tag="wv", name="wv")
        wkvh_t = pp.tile([128, KP, 2, R], F8, tag="wkvh", name="wkvh")
        wkvl_t = pp.tile([128, KP, 2, R], F8, tag="wkvl", name="wkvl")
        wqnh_t = pp.tile([128, KP, 2, HL * DN], F8, tag="wqnh", name="wqnh")
        wqnl_t = pp.tile([128, KP, 2, HL * DN], F8, tag="wqnl", name="wqnl")
        wqrh_t = pp.tile([128, KP, 2, HL * DR], F8, tag="wqrh", name="wqrh")
        wqrl_t = pp.tile([128, KP, 2, HL * DR], F8, tag="wqrl", name="wqrl")
        ct = pp.tile([128, NT, 128], BF, tag="ct", name="ct")
        st = pp.tile([128, NT, 128], BF, tag="st", name="st")
        px = outer.enter_context(tc.tile_pool(name="xp", bufs=2))
        xh_t = [None] * NJ
        xl_t = [None] * NJ

        pw = outer.enter_context(tc.tile_pool(name="work", bufs=2))
        pbig = outer.enter_context(tc.tile_pool(name="pbig", bufs=3, space="PSUM"))
        pss = outer.enter_context(tc.tile_pool(name="pss", bufs=3, space="PSUM"))
        pav = outer.enter_context(tc.tile_pool(name="pav", bufs=2, space="PSUM"))

        # ---- constants
        nc.vector.memset(epst[:], float(WS * WS * EPS))
        make_identity(nc, ident[:])
        nc.gpsimd.memset(bmask[:], 1.0)
        # keep (1) where k <= q, i.e. (-part + col) >= 0; else 0
        nc.gpsimd.affine_select(
            out=bmask[:], in_=bmask[:], compare_op=AluOpType.is_ge,
            fill=0.0, base=0, pattern=[[1, 128]], channel_multiplier=-1)
        for t in range(NT):
            nc.gpsimd.memset(Vg[t][:, :, DV:DV + 1], 1.0)

        # ---- input DMAs: first-needed first. x per (j, k-half) for overlap.
        def fetch_x(j, split=False):
            xh_t[j] = px.tile([128, KP, 2, 512], F8, tag="xh", name=f"xh{j}")
            xl_t[j] = px.tile([128, KP, 2, 512], F8, tag="xl", name=f"xl{j}")
            if split:
                nc.sync.dma_start(xh_t[j][:, 0:4], xh_d[j][:, 0:4])
                nc.sync.dma_start(xh_t[j][:, 4:8], xh_d[j][:, 4:8])
                nc.sync.dma_start(xl_t[j][:, 0:4], xl_d[j][:, 0:4])
                nc.sync.dma_start(xl_t[j][:, 4:8], xl_d[j][:, 4:8])
            else:
                nc.sync.dma_start(xh_t[j][:], xh_d[j])
                nc.sync.dma_start(xl_t[j][:], xl_d[j])

        nc.sync.dma_start(wqrh_t[:], wqrh_d[:])
        fetch_x(0, split=True)
        nc.sync.dma_start(wqrl_t[:], wqrl_d[:])
        nc.sync.dma_start(wkvh_t[:], wkvh_d[:])
        nc.sync.dma_start(wkvl_t[:], wkvl_d[:])
        nc.sync.dma_start(ct[:], cos_d[:])
        nc.sync.dma_start(st[:], sin_d[:])
        nc.sync.dma_start(wqnh_t[:], wqnh_d[:])
        nc.sync.dma_start(wqnl_t[:], wqnl_d[:])
        nc.sync.dma_start(wkn_t[:], wkn_d[:])
        nc.sync.dma_start(wkr_t[:], wkr_d[:])
        nc.sync.dma_start(wv_t[:], wv_d[:])
        for p in range(2):
            nc.sync.dma_start(woh_t[p][:], woh_d[p])
            nc.sync.dma_start(wol_t[p][:], wol_d[p])

        # ---- deferred PE work queues
        pending = []   # attention pair tails
        defer = []     # transpose+copy tails (PE transposes follow Act/DVE)
        defer_o = []   # out_proj chunks, gated on all-4-heads-emitted
        bgq = []       # background stage-A bundles for the next super
        tcnt = {}
        outT_map = {}

        def pump(n=1):
            for _ in range(min(n, len(pending))):
                pending.pop(0)()

        def drain(n=1):
            for _ in range(min(n, len(defer))):
                defer.pop(0)()

        def bg(n=1):
            for _ in range(min(n, len(bgq))):
                bgq.pop(0)()

        def drain_o(n=1):
            for _ in range(min(n, len(defer_o))):
                i = defer_o[0][0]
                while tcnt.get(i, 0) < HL:
                    if pending:
                        pending.pop(0)()
                    elif defer:
                        defer.pop(0)()
                    else:
                        break
                if tcnt.get(i, 0) < HL:
                    break
                defer_o.pop(0)[1]()

        def drain_all():
            pump(len(pending))
            bg(len(bgq))
            drain(len(defer))
            while defer_o:
                if tcnt.get(defer_o[0][0], 0) < HL:
                    pump(len(pending))
                    drain(len(defer))
                drain_o(1)

        def rope_chain(src, t, dstT):
            """src: [128,256] f32 psum (s-rows, 4 heads x 64 interleaved-pair
            rope dims). Rotates on DVE+Pool, then DMA-xbar-transposes the
            bf16 result straight into dstT[r2][:, 128t:...]."""
            rk = pw.tile([128, 256], BF, tag="rk", name="rk")
            nc.vector.tensor_copy(rk[:], src)
            rp = pw.tile([128, 256], BF, tag="rp", name="rp")
            e = rk[:, 0:256:2]
            o = rk[:, 1:256:2]
            de = rp[:, 0:256:2]
            do = rp[:, 1:256:2]
            cs = ct[:, t, :]
            sn = st[:, t, :]
            t1 = pw.tile([128, 128], BF, tag="t1", bufs=1, name="t1")
            t2 = pw.tile([128, 128], BF, tag="t2", bufs=1, name="t2")
            nc.vector.tensor_mul(t1[:], e, cs)
            nc.vector.tensor_mul(t2[:], o, sn)
            nc.vector.tensor_sub(de, t1[:], t2[:])
            t3 = pw.tile([128, 128], BF, tag="t3", bufs=1, name="t3")
            t4 = pw.tile([128, 128], BF, tag="t4", bufs=1, name="t4")
            nc.gpsimd.tensor_mul(t3[:], e, sn)
            nc.gpsimd.tensor_mul(t4[:], o, cs)
            nc.gpsimd.tensor_add(do, t3[:], t4[:])

            def emit(rp=rp, t=t, dstT=dstT):
                for r2 in range(2):
                    pt = pss.tile([128, 128], BF, tag="ss", name="ptr")
                    nc.tensor.transpose(pt[:], rp[:, 128 * r2:128 * (r2 + 1)],
                                        ident[:])
                    nc.scalar.copy(dstT[r2][:, 128 * t:128 * (t + 1)], pt[:])
            defer.append(emit)

        def rms_chain(ps, t):
            sq = pw.tile([128, 512], BF, tag="fo", bufs=4, name="sq")
            var = pw.tile([128, 1], F32, tag="var", name="var")
            nc.scalar.activation(sq[:], ps[:], AT.Square, accum_out=var[:])
            lnm = pw.tile([128, 1], F32, tag="lnm", name="lnm")
            nc.scalar.activation(lnm[:], var[:], AT.Ln,
                                 bias=epst[:], scale=1.0 / R)
            rstd = pw.tile([128, 1], F32, tag="rstd", name="rstd")
            nc.scalar.activation(rstd[:], lnm[:], AT.Exp, scale=-0.5)
            kvn = pw.tile([128, 512], BF, tag="kvn", bufs=4, name="kvn")
            nc.vector.tensor_scalar_mul(kvn[:], ps[:], rstd[:])

            def emit(kvn=kvn, t=t):
                t4 = t % 4
                for r in range(4):
                    pt = pss.tile([128, 128], BF, tag="ss", name="ptk")
                    nc.tensor.transpose(pt[:], kvn[:, 128 * r:128 * (r + 1)],
                                        ident[:])
                    nc.scalar.copy(kvT[r][:, 128 * t4:128 * (t4 + 1)], pt[:])
            defer.append(emit)

        # ---- stage A bundles (x-side projections for super j)
        def a_qr(j, t4):
            t = 4 * j + t4
            xh, xl = xh_t[j], xl_t[j]
            ssl = slice(128 * t4, 128 * (t4 + 1))
            ps = pbig.tile([128, 512], F32, tag="big", name="pqr")
            n = 0
            for (xa, wb) in ((xh, wqrh_t), (xl, wqrh_t), (xh, wqrl_t)):
                for k in range(KP):
                    nc.tensor.matmul(ps[:, 0:256], xa[:, k, :, ssl],
                                     wb[:, k, :, :],
                                     start=(n == 0), stop=(n == 3 * KP - 1),
                                     perf_mode=DRM)
                    n += 1
            drain(2)
            rope_chain(ps[:, 0:256], t, QrT)

        def a_kv(j, t4):
            t = 4 * j + t4
            xh, xl = xh_t[j], xl_t[j]
            ssl = slice(128 * t4, 128 * (t4 + 1))
            ps = pbig.tile([128, 512], F32, tag="big", name="pkv")
            n = 0
            for (xa, wb) in ((xh, wkvh_t), (xl, wkvh_t), (xh, wkvl_t)):
                for k in range(KP):
                    nc.tensor.matmul(ps[:], xa[:, k, :, ssl],
                                     wb[:, k, :, :],
                                     start=(n == 0), stop=(n == 3 * KP - 1),
                                     perf_mode=DRM)
                    n += 1
            drain(4)
            drain_o(1)
            rms_chain(ps, t)

        def a_qn(j, h):
            xh, xl = xh_t[j], xl_t[j]
            hsl = slice(128 * h, 128 * (h + 1))
            jsl = slice(512 * j, 512 * (j + 1))
            ps = pbig.tile([128, 512], F32, tag="big", name="pqn")
            n = 0
            for k in range(KP):
                for (wa, xb) in ((wqnh_t, xh), (wqnl_t, xh), (wqnh_t, xl)):
                    nc.tensor.matmul(ps[:], wa[:, k, :, hsl],
                                     xb[:, k, :, :],
                                     start=(n == 0), stop=(n == 3 * KP - 1),
                                     perf_mode=DRM)
                    n += 1
                if k == 3:
                    drain(2)
                    drain_o(1)
            nc.vector.tensor_copy(QnT[h][:, jsl], ps[:])

        # ---- stage B bundles (latent up-projections)
        def b_kr(j, t4):
            t = 4 * j + t4
            tsl = slice(128 * t4, 128 * (t4 + 1))
            ps = pbig.tile([128, 512], F32, tag="big", name="pkr")
            for r in range(4):
                nc.tensor.matmul(ps[:, 0:256], kvT[r][:, tsl],
                                 wkr_t[:, r, :], start=(r == 0),
                                 stop=(r == 3))
            drain(1)
            rope_chain(ps[:, 0:256], t, KrT)

        def b_kn(j, h):
            hsl = slice(128 * h, 128 * (h + 1))
            jsl = slice(512 * j, 512 * (j + 1))
            ps = pbig.tile([128, 512], F32, tag="big", name="pkn")
            for r in range(4):
                nc.tensor.matmul(ps[:], wkn_t[:, r, hsl], kvT[r][:],
                                 start=(r == 0), stop=(r == 3))
            drain(1)
            nc.vector.tensor_copy(KnT[h][:, jsl], ps[:])

        def b_v(j, t4):
            t = 4 * j + t4
            tsl = slice(128 * t4, 128 * (t4 + 1))
            psv = pbig.tile([128, 512], F32, tag="big", name="pv")
            for r in range(4):
                nc.tensor.matmul(psv[:], kvT[r][:, tsl], wv_t[:, r, :],
                                 start=(r == 0), stop=(r == 3))
            drain(1)
            nc.vector.tensor_copy(Vg[t][:, :, 0:DV], psv[:])

        # ---- attention
        def _new_st(i, h):
            return {"h": h, "av": pav.tile([128, 132], F32, tag="av", name="av"),
                    "first": True, "avq": []}

        def _pop_av(st, n, last=False):
            for _ in range(min(n, len(st["avq"]))):
                K, ptsl = st["avq"].pop(0)
                fin = last and not st["avq"]
                nc.tensor.matmul(st["av"][:, 0:DV + 1], ptsl,
                                 Vg[K][:, st["h"], :],
                                 start=st["first"], stop=fin)
                st["first"] = False

        def _emit_group(i, st, k0, gsz):
            h = st["h"]
            ro = 64 * (h % 2)
            qsl = slice(128 * i, 128 * (i + 1))
            ps = pss.tile([128, 512], F32, tag="ss", name="ss")
            for u in range(gsz):
                K = k0 + u
                csl = slice(128 * u, 128 * (u + 1))
                ksl = slice(128 * K, 128 * (K + 1))
                nc.tensor.matmul(ps[:, csl], KnT[h][:, ksl], QnT[h][:, qsl],
                                 start=True, stop=False)
                nc.tensor.matmul(ps[:, csl], KrT[h // 2][ro:ro + 64, ksl],
                                 QrT[h // 2][ro:ro + 64, qsl],
                                 start=False, stop=True)
                if len(st["avq"]) > 7:  # ~4-group skew behind the exps
                    _pop_av(st, 1)
                if u == 1:
                    drain(1)
                elif u & 1:
                    drain_o(1)
            pt = pw.tile([128, 512], BF, tag="pT", bufs=8, name="pt")
            w = 128 * gsz
            nc.scalar.activation(pt[:, 0:w], ps[:, 0:w], AT.Exp, scale=SCALE)
            if k0 + gsz == i + 1:  # zero masked-out weights on diagonal unit
                dsl = slice(128 * (gsz - 1), 128 * gsz)
                nc.vector.tensor_mul(pt[:, dsl], pt[:, dsl], bmask[:])
            for u in range(gsz):
                st["avq"].append((k0 + u, pt[:, 128 * u:128 * (u + 1)]))

        def _tail(i, st):
            _pop_av(st, len(st["avq"]), last=True)
            h = st["h"]
            av = st["av"]
            rinv = pw.tile([128, 1], F32, tag="rinv", bufs=4, name="rinv")
            nc.vector.reciprocal_approx_fast(rinv[:], av[:, DV:DV + 1])
            oq = pw.tile([128, 128], BF, tag="oq", bufs=6, name="oq")
            nc.vector.tensor_scalar_mul(oq[:], av[:, 0:DV], rinv[:])

            def emit(i=i, h=h, oq=oq):
                pt = pss.tile([128, 128], BF, tag="ss", name="pto")
                nc.tensor.transpose(pt[:], oq[:], ident[:])
                if h == 0:
                    outT_map[i] = (
                        pw.tile([128, HL, 128], F8, tag="oth", bufs=2,
                                name="oth"),
                        pw.tile([128, HL, 128], F8, tag="otl", bufs=2,
                                name="otl"))
                oth, otl = outT_map[i]
                nc.vector.tensor_copy(oth[:, h, :], pt[:])
                nc.vector.tensor_sub(otl[:, h, :], pt[:], oth[:, h, :])
                tcnt[i] = tcnt.get(i, 0) + 1
            defer.append(emit)

        def attn_pair(i, h0, h1):
            sts = [_new_st(i, h0), _new_st(i, h1)]
            nu = i + 1
            first = True
            for k0 in range(0, nu, 4):
                gsz = min(4, nu - k0)
                for st in sts:
                    _emit_group(i, st, k0, gsz)
                if first:  # previous pair's tails overlap our first group
                    pump(2)
                first = False
            pending.append(lambda: _tail(i, sts[0]))
            pending.append(lambda: _tail(i, sts[1]))

        def op_chunk(i, dsl):
            ps = pbig.tile([128, 512], F32, tag="big", name="psf")
            dcs = slice(512 * dsl, 512 * (dsl + 1))
            isl = slice(128 * i, 128 * (i + 1))
            oth, otl = outT_map[i]
            n = 0
            for p in range(2):
                psl = slice(2 * p, 2 * p + 2)
                for (oa, wb) in ((oth, woh_t[p]), (otl, woh_t[p]),
                                 (oth, wol_t[p])):
                    nc.tensor.matmul(ps[:], oa[:, psl, :], wb[:, :, dcs],
                                     start=(n == 0), stop=(n == 5),
                                     perf_mode=DRM)
                    n += 1
            fo = pw.tile([128, 512], BF, tag="fo", bufs=4, name="fo")
            nc.vector.tensor_copy(fo[:], ps[:])
            nc.sync.dma_start(out_d[isl, dcs], fo[:])

        # ================= per-super pipeline =================
        for j in range(NJ):
            if j + 1 < NJ:
                fetch_x(j + 1)
            # stage A: q_rope (rope chains overlap the rest), kv latent, q_nope
            for t4 in range(4):
                a_qr(j, t4)
            for t4 in range(4):
                a_kv(j, t4)
            for h in range(HL):
                a_qn(j, h)

            # stage B: k_rope first so its rope chains overlap kn/v matmuls
            drain_all()
            for t4 in range(4):
                b_kr(j, t4)
            for h in range(HL):
                b_kn(j, h)
            for t4 in range(4):
                b_v(j, t4)

            # stage C: attention + out_proj for q-tiles of this super
            drain_all()
            for t4 in range(4):
                i = 4 * j + t4
                attn_pair(i, 0, 1)
                attn_pair(i, 2, 3)
                for dsl in range(4):
                    defer_o.append((i, lambda i=i, dsl=dsl: op_chunk(i, dsl)))
        drain_all()

    nc.compile()
    return nc


def _f8_hilo(a):
    f8 = ml_dtypes.float8_e4m3
    hi = a.astype(f8)
    lo = (a - hi.astype(np.float32)).astype(f8)
    return hi, lo


def _prep_inputs(x, freqs, w_kv, g_kv, w_k, w_v, w_qn, w_qr, w_o):
    bf = ml_dtypes.bfloat16
    f32 = np.float32

    def pack_x(a):  # [D, S] -> [NJ, 128, KP, 2, 512]
        return np.ascontiguousarray(
            a.reshape(KP, 2, 128, NJ, 512).transpose(3, 2, 0, 1, 4))

    def pack_w(a):  # [D, C] -> [128, KP, 2, C]
        return np.ascontiguousarray(
            a.reshape(KP, 2, 128, a.shape[1]).transpose(2, 0, 1, 3))

    def pack_r(a):  # [R, C] -> [128, 4, C]
        return np.ascontiguousarray(
            a.reshape(4, 128, a.shape[1]).transpose(1, 0, 2))

    wk3 = (w_k.astype(f32) * g_kv.astype(f32)[:, None]).reshape(R, H, DN + DR)
    wv2 = (w_v.astype(f32) * g_kv.astype(f32)[:, None]).reshape(R, H, DV)

    # rope tables: packed e-view col c (of 128) has angle freqs[s, c % 32]
    ang = freqs.astype(f32)  # [S, 32]
    idx = np.tile(np.arange(32), 4)
    cos4 = np.cos(ang)[:, idx].reshape(NT, 128, 128).transpose(1, 0, 2)
    sin4 = np.sin(ang)[:, idx].reshape(NT, 128, 128).transpose(1, 0, 2)
    cosb = np.ascontiguousarray(cos4).astype(bf)
    sinb = np.ascontiguousarray(sin4).astype(bf)

    wkvh, wkvl = _f8_hilo(w_kv.astype(f32) * WS)
    in_maps = []
    for c in range(8):
        b, g = c // 4, c % 4
        hs = slice(4 * g, 4 * g + 4)
        xT = np.ascontiguousarray(x[b].astype(f32).T)  # [D, S]
        xh, xl = _f8_hilo(xT)
        wqn_c = np.ascontiguousarray(
            w_qn.reshape(D, H, DN)[:, hs].reshape(D, HL * DN)).astype(f32) * WS
        wqr_c = np.ascontiguousarray(
            w_qr.reshape(D, H, DR)[:, hs].reshape(D, HL * DR)).astype(f32) * WS
        wqnh, wqnl = _f8_hilo(wqn_c)
        wqrh, wqrl = _f8_hilo(wqr_c)
        m = {
            "xh": pack_x(xh), "xl": pack_x(xl),
            "wkvh": pack_w(wkvh), "wkvl": pack_w(wkvl),
            "wqnh": pack_w(wqnh), "wqnl": pack_w(wqnl),
            "wqrh": pack_w(wqrh), "wqrl": pack_w(wqrl),
            "wkn": pack_r(np.ascontiguousarray(
                wk3[:, hs, :DN].reshape(R, HL * DN)).astype(bf)),
            "wkr": pack_r(np.ascontiguousarray(
                wk3[:, hs, DN:].reshape(R, HL * DR)).astype(bf)),
            "wv": pack_r(np.ascontiguousarray(
                wv2[:, hs].reshape(R, HL * DV)).astype(bf)),
            "cosb": cosb, "sinb": sinb,
        }
        wo4 = w_o.reshape(H, DV, D)[hs].astype(f32) * WS
        f8t = ml_dtypes.float8_e4m3
        m["woh"] = np.empty((2, 128, 2, D), f8t)
        m["wol"] = np.empty((2, 128, 2, D), f8t)
        for p in range(2):
            wop = np.ascontiguousarray(
                wo4[2 * p:2 * p + 2].transpose(1, 0, 2))
            m["woh"][p], m["wol"][p] = _f8_hilo(wop)
        in_maps.append(m)
    return in_maps


def kernel(x, freqs, w_kv, g_kv, w_k, w_v, w_qn, w_qr, w_o):
    if "nc" not in _CACHE:
        _CACHE["nc"] = _build()
    nc = _CACHE["nc"]
    in_maps = _prep_inputs(np.asarray(x), np.asarray(freqs), np.asarray(w_kv),
                           np.asarray(g_kv), np.asarray(w_k), np.asarray(w_v),
                           np.asarray(w_qn), np.asarray(w_qr), np.asarray(w_o))
    res = run_bass_kernel_spmd(nc, in_maps, list(range(8)), trace=False)
    out = np.zeros((B, S, D), np.float32)
    for c in range(8):
        out[c // 4] += res.results[c]["out"].astype(np.float32)
    out /= WS  # out_proj weights were pre-scaled for fp8
    return out


# revision 28
# speedup vs baseline: 1.0057x; 1.0057x over previous
"""MLA forward kernel for Trainium2, 8 NeuronCores.

Sharding: data-parallel over batch (2) x tensor-parallel over heads (16 -> 4
groups of 4). Core c handles batch c//4, head group c%4. kv compression is
replicated per core. Each core emits a partial [S, D] output (its heads'
contribution through out_proj) in bf16; the host sums the 4 partials per batch
in fp32.

Key design points:
  - x-side projections (kv latent, q_nope, q_rope) in fp8e4 hi/lo DoubleRow
    matmuls (3 products), weights pre-scaled by 32. x packed [128, KP, 2, 512]
    per 512-row super so q_nope uses a 512-wide moving operand and comes out
    head-major (no transpose); k_nope likewise 512 wide per (head, super).
  - one activation table for the whole kernel ({Square, Ln, Exp, Copy},
    pinned via _pin_act_tables): rmsnorm rstd = Exp(-0.5*Ln(var/R + eps)) on
    Act. This avoids ~33 LoadActFuncSet swaps (1.3us each) that bass's greedy
    per-instruction table choice would otherwise emit.
  - kv latent is kept one super deep ([128,512] ring): stage B of super j
    consumes it before stage A of super j+1 rewrites it (PE program order).
  - attention: scores^T per 128x128 block (exact causal block-triangle),
    P^T = exp(scale*s) bf16; AV fused with the softmax row-sum via a
    ones-column on V; normalization = reciprocal_approx_fast + per-partition
    tensor_scalar mul. Deep pt/fo/oq rings keep the exp->AV->out_proj
    pipeline fed.
  - rope rotations split across DVE and Pool from a bf16 SBUF copy of the
    psum tile; transposes on PE via deferred-emission queues that fill PE
    dependency stalls (rope/kv/attention-output transposes + out_proj chunks).
  - out_proj in fp8 hi/lo DoubleRow (head-pairs packed), host divides by 32.
"""

import sys
import numpy as np
import ml_dtypes

sys.path.insert(0, "/opt/trn_rl_repo")

import concourse.bass as bass  # noqa: E402
import concourse.tile as tile  # noqa: E402
from concourse import mybir, bacc  # noqa: E402
from concourse.bass_utils import run_bass_kernel_spmd  # noqa: E402
from concourse.masks import make_identity  # noqa: E402
from concourse.alu_op_type import AluOpType  # noqa: E402
from contextlib import ExitStack  # noqa: E402

B, S, D = 2, 2048, 2048
H, DN, DR, DV, R = 16, 128, 64, 128, 512
HL = 4  # heads per core
EPS = 1e-6
WS = 32.0  # fp8 weight pre-scale
SCALE = 1.0 / (float(np.sqrt(DN + DR)) * WS)
BF = mybir.dt.bfloat16
F32 = mybir.dt.float32
F8 = mybir.dt.float8e4
NT = S // 128   # 16 s-tiles
NJ = 4          # s-supers of 512
KP = 8          # D packs of 256
DRM = mybir.MatmulPerfMode.DoubleRow
AT = mybir.ActivationFunctionType

_CACHE = {}


def _pin_act_tables():
    """Route every activation to the one table set holding all four
    functions we use ({Square, Ln, Exp, Copy}); otherwise bass's greedy
    per-instruction set choice thrashes LoadActFuncSet ~33x per kernel.
    Indices are preserved (only membership is masked) so the emitted
    act_func_set_id still matches act_info.json."""
    import concourse.bacc as _bacc_mod
    if getattr(_bacc_mod, "_act_tables_pinned", False):
        return
    orig = _bacc_mod.get_activation_tables
    keep = "natural_log_exp_and_others"

    def pinned(arch):
        t = orig(arch)
        return {k: (v if k == keep else type(v)()) for k, v in t.items()}

    _bacc_mod.get_activation_tables = pinned
    _bacc_mod._act_tables_pinned = True


def _build():
    _pin_act_tables()
    nc = bacc.Bacc("TRN2", target_bir_lowering=False, debug=False)

    def din(name, shape, dt):
        return nc.dram_tensor(name, list(shape), dt, kind="ExternalInput").ap()

    xh_d = din("xh", [NJ, 128, KP, 2, 512], F8)
    xl_d = din("xl", [NJ, 128, KP, 2, 512], F8)
    wkvh_d = din("wkvh", [128, KP, 2, R], F8)
    wkvl_d = din("wkvl", [128, KP, 2, R], F8)
    wqnh_d = din("wqnh", [128, KP, 2, HL * DN], F8)
    wqnl_d = din("wqnl", [128, KP, 2, HL * DN], F8)
    wqrh_d = din("wqrh", [128, KP, 2, HL * DR], F8)
    wqrl_d = din("wqrl", [128, KP, 2, HL * DR], F8)
    wkn_d = din("wkn", [128, 4, HL * DN], BF)
    wkr_d = din("wkr", [128, 4, HL * DR], BF)
    wv_d = din("wv", [128, 4, HL * DV], BF)
    woh_d = din("woh", [2, 128, 2, D], F8)
    wol_d = din("wol", [2, 128, 2, D], F8)
    cos_d = din("cosb", [128, NT, 128], BF)
    sin_d = din("sinb", [128, NT, 128], BF)
    out_d = nc.dram_tensor("out", [S, D], BF, kind="ExternalOutput").ap()

    with tile.TileContext(nc) as tc, ExitStack() as outer:
        pp = outer.enter_context(tc.tile_pool(name="persist", bufs=1))
        ident = pp.tile([128, 128], BF, tag="ident", name="ident")
        bmask = pp.tile([128, 128], BF, tag="bmask", name="bmask")
        epst = pp.tile([128, 1], F32, tag="epst", name="epst")
        QnT = [pp.tile([128, S], BF, tag=f"QnT{h}", name=f"QnT{h}")
               for h in range(HL)]
        KnT = [pp.tile([128, S], BF, tag=f"KnT{h}", name=f"KnT{h}")
               for h in range(HL)]
        QrT = [pp.tile([128, S], BF, tag=f"QrT{r}", name=f"QrT{r}")
               for r in range(2)]
        KrT = [pp.tile([128, S], BF, tag=f"KrT{r}", name=f"KrT{r}")
               for r in range(2)]
        # kv latent, one super deep (stage B of super j consumes it before
        # stage A of super j+1 rewrites it; PE program order guarantees this)
        kvT = [pp.tile([128, 512], BF, tag=f"kvT{r}", name=f"kvT{r}")
               for r in range(4)]
        Vg = [pp.tile([128, HL, DV + 1], BF, tag=f"Vg{t}", name=f"Vg{t}")
              for t in range(NT)]
        woh_t = [pp.tile([128, 2, D], F8, tag=f"woh{p}", name=f"woh{p}")
                 for p in range(2)]
        wol_t = [pp.tile([128, 2, D], F8, tag=f"wol{p}", name=f"wol{p}")
                 for p in range(2)]
        wkn_t = pp.tile([128, 4, HL * DN], BF, tag="wkn", name="wkn")
        wkr_t = pp.tile([128, 4, HL * DR], BF, tag="wkr", name="wkr")
        wv_t = pp.tile([128, 4, HL * DV], BF, tag="wv", name="wv")
        wkvh_t = pp.tile([128, KP, 2, R], F8, tag="wkvh", name="wkvh")
        wkvl_t = pp.tile([128, KP, 2, R], F8, tag="wkvl", name="wkvl")
        wqnh_t = pp.tile([128, KP, 2, HL * DN], F8, tag="wqnh", name="wqnh")
        wqnl_t = pp.tile([128, KP, 2, HL * DN], F8, tag="wqnl", name="wqnl")
        wqrh_t = pp.tile([128, KP, 2, HL * DR], F8, tag="wqrh", name="wqrh")
        wqrl_t = pp.tile([128, KP, 2, HL * DR], F8, tag="wqrl", name="wqrl")
        ct = pp.tile([128, NT, 128], BF, tag="ct", name="ct")
        st = pp.tile([128, NT, 128], BF, tag="st", name="st")
        px = outer.enter_context(tc.tile_pool(name="xp", bufs=2))
        xh_t = [None] * NJ
        xl_t = [None] * NJ

        pw = outer.enter_context(tc.tile_pool(name="work", bufs=2))
        pbig = outer.enter_context(tc.tile_pool(name="pbig", bufs=3, space="PSUM"))
        pss = outer.enter_context(tc.tile_pool(name="pss", bufs=3, space="PSUM"))
        pav = outer.enter_context(tc.tile_pool(name="pav", bufs=2, space="PSUM"))

        # ---- constants
        nc.vector.memset(epst[:], float(WS * WS * EPS))
        make_identity(nc, ident[:])
        nc.gpsimd.memset(bmask[:], 1.0)
        # keep (1) where k <= q, i.e. (-part + col) >= 0; else 0
        nc.gpsimd.affine_select(
            out=bmask[:], in_=bmask[:], compare_op=AluOpType.is_ge,
            fill=0.0, base=0, pattern=[[1, 128]], channel_multiplier=-1)
        for t in range(NT):
            nc.gpsimd.memset(Vg[t][:, :, DV:DV + 1], 1.0)

        # ---- input DMAs: first-needed first. x per (j, k-half) for overlap.
        def fetch_x(j, split=False):
            xh_t[j] = px.tile([128, KP, 2, 512], F8, tag="xh", name=f"xh{j}")
            xl_t[j] = px.tile([128, KP, 2, 512], F8, tag="xl", name=f"xl{j}")
            if split:
                nc.sync.dma_start(xh_t[j][:, 0:4], xh_d[j][:, 0:4])
                nc.sync.dma_start(xh_t[j][:, 4:8], xh_d[j][:, 4:8])
                nc.sync.dma_start(xl_t[j][:, 0:4], xl_d[j][:, 0:4])
                nc.sync.dma_start(xl_t[j][:, 4:8], xl_d[j][:, 4:8])
            else:
                nc.sync.dma_start(xh_t[j][:], xh_d[j])
                nc.sync.dma_start(xl_t[j][:], xl_d[j])

        nc.sync.dma_start(wqrh_t[:], wqrh_d[:])
        fetch_x(0, split=True)
        nc.sync.dma_start(wqrl_t[:], wqrl_d[:])
        nc.sync.dma_start(wkvh_t[:], wkvh_d[:])
        nc.sync.dma_start(wkvl_t[:], wkvl_d[:])
        nc.sync.dma_start(ct[:], cos_d[:])
        nc.sync.dma_start(st[:], sin_d[:])
        nc.sync.dma_start(wqnh_t[:], wqnh_d[:])
        nc.sync.dma_start(wqnl_t[:], wqnl_d[:])
        nc.sync.dma_start(wkn_t[:], wkn_d[:])
        nc.sync.dma_start(wkr_t[:], wkr_d[:])
        nc.sync.dma_start(wv_t[:], wv_d[:])
        for p in range(2):
            nc.sync.dma_start(woh_t[p][:], woh_d[p])
            nc.sync.dma_start(wol_t[p][:], wol_d[p])

        # ---- deferred PE work queues
        pending = []   # attention pair tails
        defer = []     # transpose+copy tails (PE transposes follow Act/DVE)
        defer_o = []   # out_proj chunks, gated on all-4-heads-emitted
        bgq = []       # background stage-A bundles for the next super
        tcnt = {}
        outT_map = {}

        def pump(n=1):
            for _ in range(min(n, len(pending))):
                pending.pop(0)()

        def drain(n=1):
            for _ in range(min(n, len(defer))):
                defer.pop(0)()

        def bg(n=1):
            for _ in range(min(n, len(bgq))):
                bgq.pop(0)()

        def drain_o(n=1):
            for _ in range(min(n, len(defer_o))):
                i = defer_o[0][0]
                while tcnt.get(i, 0) < HL:
                    if pending:
                        pending.pop(0)()
                    elif defer:
                        defer.pop(0)()
                    else:
                        break
                if tcnt.get(i, 0) < HL:
                    break
                defer_o.pop(0)[1]()

        def drain_all():
            pump(len(pending))
            bg(len(bgq))
            drain(len(defer))
            while defer_o:
                if tcnt.get(defer_o[0][0], 0) < HL:
                    pump(len(pending))
                    drain(len(defer))
                drain_o(1)

        def rope_chain(src, t, dstT):
            """src: [128,256] f32 psum (s-rows, 4 heads x 64 interleaved-pair
            rope dims). Rotates on DVE+Pool, then DMA-xbar-transposes the
            bf16 result straight into dstT[r2][:, 128t:...]."""
            rk = pw.tile([128, 256], BF, tag="rk", name="rk")
            nc.vector.tensor_copy(rk[:], src)
            rp = pw.tile([128, 256], BF, tag="rp", name="rp")
            e = rk[:, 0:256:2]
            o = rk[:, 1:256:2]
            de = rp[:, 0:256:2]
            do = rp[:, 1:256:2]
            cs = ct[:, t, :]
            sn = st[:, t, :]
            t1 = pw.tile([128, 128], BF, tag="t1", bufs=1, name="t1")
            t2 = pw.tile([128, 128], BF, tag="t2", bufs=1, name="t2")
            nc.vector.tensor_mul(t1[:], e, cs)
            nc.vector.tensor_mul(t2[:], o, sn)
            nc.vector.tensor_sub(de, t1[:], t2[:])
            t3 = pw.tile([128, 128], BF, tag="t3", bufs=1, name="t3")
            t4 = pw.tile([128, 128], BF, tag="t4", bufs=1, name="t4")
            nc.gpsimd.tensor_mul(t3[:], e, sn)
            nc.gpsimd.tensor_mul(t4[:], o, cs)
            nc.gpsimd.tensor_add(do, t3[:], t4[:])

            def emit(rp=rp, t=t, dstT=dstT):
                for r2 in range(2):
                    pt = pss.tile([128, 128], BF, tag="ss", name="ptr")
                    nc.tensor.transpose(pt[:], rp[:, 128 * r2:128 * (r2 + 1)],
                                        ident[:])
                    nc.scalar.copy(dstT[r2][:, 128 * t:128 * (t + 1)], pt[:])
            defer.append(emit)

        def rms_chain(ps, t):
            sq = pw.tile([128, 512], BF, tag="fo", bufs=4, name="sq")
            var = pw.tile([128, 1], F32, tag="var", name="var")
            nc.scalar.activation(sq[:], ps[:], AT.Square, accum_out=var[:])
            lnm = pw.tile([128, 1], F32, tag="lnm", name="lnm")
            nc.scalar.activation(lnm[:], var[:], AT.Ln,
                                 bias=epst[:], scale=1.0 / R)
            rstd = pw.tile([128, 1], F32, tag="rstd", name="rstd")
            nc.scalar.activation(rstd[:], lnm[:], AT.Exp, scale=-0.5)
            kvn = pw.tile([128, 512], BF, tag="kvn", bufs=3, name="kvn")
            nc.vector.tensor_scalar_mul(kvn[:], ps[:], rstd[:])

            def emit(kvn=kvn, t=t):
                t4 = t % 4
                for r in range(4):
                    pt = pss.tile([128, 128], BF, tag="ss", name="ptk")
                    nc.tensor.transpose(pt[:], kvn[:, 128 * r:128 * (r + 1)],
                                        ident[:])
                    nc.scalar.copy(kvT[r][:, 128 * t4:128 * (t4 + 1)], pt[:])
            defer.append(emit)

        # ---- stage A bundles (x-side projections for super j)
        def a_qr(j, t4):
            t = 4 * j + t4
            xh, xl = xh_t[j], xl_t[j]
            ssl = slice(128 * t4, 128 * (t4 + 1))
            ps = pbig.tile([128, 512], F32, tag="big", name="pqr")
            n = 0
            for (xa, wb) in ((xh, wqrh_t), (xl, wqrh_t), (xh, wqrl_t)):
                for k in range(KP):
                    nc.tensor.matmul(ps[:, 0:256], xa[:, k, :, ssl],
                                     wb[:, k, :, :],
                                     start=(n == 0), stop=(n == 3 * KP - 1),
                                     perf_mode=DRM)
                    n += 1
            drain(2)
            rope_chain(ps[:, 0:256], t, QrT)

        def a_kv(j, t4):
            t = 4 * j + t4
            xh, xl = xh_t[j], xl_t[j]
            ssl = slice(128 * t4, 128 * (t4 + 1))
            ps = pbig.tile([128, 512], F32, tag="big", name="pkv")
            n = 0
            for (xa, wb) in ((xh, wkvh_t), (xl, wkvh_t), (xh, wkvl_t)):
                for k in range(KP):
                    nc.tensor.matmul(ps[:], xa[:, k, :, ssl],
                                     wb[:, k, :, :],
                                     start=(n == 0), stop=(n == 3 * KP - 1),
                                     perf_mode=DRM)
                    n += 1
            drain(4)
            drain_o(1)
            rms_chain(ps, t)

        def a_qn(j, h):
            xh, xl = xh_t[j], xl_t[j]
            hsl = slice(128 * h, 128 * (h + 1))
            jsl = slice(512 * j, 512 * (j + 1))
            ps = pbig.tile([128, 512], F32, tag="big", name="pqn")
            n = 0
            for k in range(KP):
                for (wa, xb) in ((wqnh_t, xh), (wqnl_t, xh), (wqnh_t, xl)):
                    nc.tensor.matmul(ps[:], wa[:, k, :, hsl],
                                     xb[:, k, :, :],
                                     start=(n == 0), stop=(n == 3 * KP - 1),
                                     perf_mode=DRM)
                    n += 1
                if k == 3:
                    drain(2)
                    drain_o(1)
            nc.vector.tensor_copy(QnT[h][:, jsl], ps[:])

        # ---- stage B bundles (latent up-projections)
        def b_kr(j, t4):
            t = 4 * j + t4
            tsl = slice(128 * t4, 128 * (t4 + 1))
            ps = pbig.tile([128, 512], F32, tag="big", name="pkr")
            for r in range(4):
                nc.tensor.matmul(ps[:, 0:256], kvT[r][:, tsl],
                                 wkr_t[:, r, :], start=(r == 0),
                                 stop=(r == 3))
            drain(1)
            rope_chain(ps[:, 0:256], t, KrT)

        def b_kn(j, h):
            hsl = slice(128 * h, 128 * (h + 1))
            jsl = slice(512 * j, 512 * (j + 1))
            ps = pbig.tile([128, 512], F32, tag="big", name="pkn")
            for r in range(4):
                nc.tensor.matmul(ps[:], wkn_t[:, r, hsl], kvT[r][:],
                                 start=(r == 0), stop=(r == 3))
            drain(1)
            nc.vector.tensor_copy(KnT[h][:, jsl], ps[:])

        def b_v(j, t4):
            t = 4 * j + t4
            tsl = slice(128 * t4, 128 * (t4 + 1))
            psv = pbig.tile([128, 512], F32, tag="big", name="pv")
            for r in range(4):
                nc.tensor.matmul(psv[:], kvT[r][:, tsl], wv_t[:, r, :],
                                 start=(r == 0), stop=(r == 3))
            drain(1)
            nc.vector.tensor_copy(Vg[t][:, :, 0:DV], psv[:])

        # ---- attention
        def _new_st(i, h):
            return {"h": h, "av": pav.tile([128, 132], F32, tag="av", name="av"),
                    "first": True, "avq": []}

        def _pop_av(st, n, last=False):
            for _ in range(min(n, len(st["avq"]))):
                K, ptsl = st["avq"].pop(0)
                fin = last and not st["avq"]
                nc.tensor.matmul(st["av"][:, 0:DV + 1], ptsl,
                                 Vg[K][:, st["h"], :],
                                 start=st["first"], stop=fin)
                st["first"] = False

        def _emit_group(i, st, k0, gsz):
            h = st["h"]
            ro = 64 * (h % 2)
            qsl = slice(128 * i, 128 * (i + 1))
            ps = pss.tile([128, 512], F32, tag="ss", name="ss")
            for u in range(gsz):
                K = k0 + u
                csl = slice(128 * u, 128 * (u + 1))
                ksl = slice(128 * K, 128 * (K + 1))
                nc.tensor.matmul(ps[:, csl], KnT[h][:, ksl], QnT[h][:, qsl],
                                 start=True, stop=False)
                nc.tensor.matmul(ps[:, csl], KrT[h // 2][ro:ro + 64, ksl],
                                 QrT[h // 2][ro:ro + 64, qsl],
                                 start=False, stop=True)
                if len(st["avq"]) > 5:  # ~3-group skew behind the exps
                    _pop_av(st, 1)
                if u == 1:
                    drain(1)
                elif u & 1:
                    drain_o(2)
            pt = pw.tile([128, 512], BF, tag="pT", bufs=8, name="pt")
            w = 128 * gsz
            nc.scalar.activation(pt[:, 0:w], ps[:, 0:w], AT.Exp, scale=SCALE)
            if k0 + gsz == i + 1:  # zero masked-out weights on diagonal unit
                dsl = slice(128 * (gsz - 1), 128 * gsz)
                nc.vector.tensor_mul(pt[:, dsl], pt[:, dsl], bmask[:])
            for u in range(gsz):
                st["avq"].append((k0 + u, pt[:, 128 * u:128 * (u + 1)]))

        def _tail(i, st):
            _pop_av(st, len(st["avq"]), last=True)
            h = st["h"]
            av = st["av"]
            rinv = pw.tile([128, 1], F32, tag="rinv", bufs=4, name="rinv")
            nc.vector.reciprocal_approx_fast(rinv[:], av[:, DV:DV + 1])
            oq = pw.tile([128, 128], BF, tag="oq", bufs=6, name="oq")
            nc.vector.tensor_scalar_mul(oq[:], av[:, 0:DV], rinv[:])

            def emit(i=i, h=h, oq=oq):
                pt = pss.tile([128, 128], BF, tag="ss", name="pto")
                nc.tensor.transpose(pt[:], oq[:], ident[:])
                if h == 0:
                    outT_map[i] = (
                        pw.tile([128, HL, 128], F8, tag="oth", bufs=2,
                                name="oth"),
                        pw.tile([128, HL, 128], F8, tag="otl", bufs=2,
                                name="otl"))
                oth, otl = outT_map[i]
                nc.vector.tensor_copy(oth[:, h, :], pt[:])
                nc.vector.tensor_sub(otl[:, h, :], pt[:], oth[:, h, :])
                tcnt[i] = tcnt.get(i, 0) + 1
            defer.append(emit)

        def attn_pair(i, h0, h1):
            sts = [_new_st(i, h0), _new_st(i, h1)]
            nu = i + 1
            first = True
            for k0 in range(0, nu, 4):
                gsz = min(4, nu - k0)
                for st in sts:
                    _emit_group(i, st, k0, gsz)
                if first:  # previous pair's tails overlap our first group
                    pump(4)
                first = False
            pending.append(lambda: _tail(i, sts[0]))
            pending.append(lambda: _tail(i, sts[1]))

        def op_chunk(i, dsl):
            ps = pbig.tile([128, 512], F32, tag="big", name="psf")
            dcs = slice(512 * dsl, 512 * (dsl + 1))
            isl = slice(128 * i, 128 * (i + 1))
            oth, otl = outT_map[i]
            n = 0
            for p in range(2):
                psl = slice(2 * p, 2 * p + 2)
                for (oa, wb) in ((oth, woh_t[p]), (otl, woh_t[p]),
                                 (oth, wol_t[p])):
                    nc.tensor.matmul(ps[:], oa[:, psl, :], wb[:, :, dcs],
                                     start=(n == 0), stop=(n == 5),
                                     perf_mode=DRM)
                    n += 1
            fo = pw.tile([128, 512], BF, tag="fo", bufs=4, name="fo")
            nc.vector.tensor_copy(fo[:], ps[:])
            nc.sync.dma_start(out_d[isl, dcs], fo[:])

        # ================= per-super pipeline =================
        for j in range(NJ):
            if j + 1 < NJ:
                fetch_x(j + 1)
            # stage A: q_rope (rope chains overlap the rest), kv latent, q_nope
            for t4 in range(4):
                a_qr(j, t4)
            for t4 in range(4):
                a_kv(j, t4)
            for h in range(HL):
                a_qn(j, h)

            # stage B: k_rope first so its rope chains overlap kn/v matmuls
            drain_all()
            for t4 in range(4):
                b_kr(j, t4)
            for h in range(HL):
                b_kn(j, h)
            for t4 in range(4):
                b_v(j, t4)

            # stage C: attention + out_proj for q-tiles of this super
            drain_all()
            for t4 in range(4):
                i = 4 * j + t4
                attn_pair(i, 0, 1)
                attn_pair(i, 2, 3)
                for dsl in range(4):
                    defer_o.append((i, lambda i=i, dsl=dsl: op_chunk(i, dsl)))
        drain_all()

    nc.compile()
    return nc


def _f8_hilo(a):
    f8 = ml_dtypes.float8_e4m3
    hi = a.astype(f8)
    lo = (a - hi.astype(np.float32)).astype(f8)
    return hi, lo


def _prep_inputs(x, freqs, w_kv, g_kv, w_k, w_v, w_qn, w_qr, w_o):
    bf = ml_dtypes.bfloat16
    f32 = np.float32

    def pack_x(a):  # [D, S] -> [NJ, 128, KP, 2, 512]
        return np.ascontiguousarray(
            a.reshape(KP, 2, 128, NJ, 512).transpose(3, 2, 0, 1, 4))

    def pack_w(a):  # [D, C] -> [128, KP, 2, C]
        return np.ascontiguousarray(
            a.reshape(KP, 2, 128, a.shape[1]).transpose(2, 0, 1, 3))

    def pack_r(a):  # [R, C] -> [128, 4, C]
        return np.ascontiguousarray(
            a.reshape(4, 128, a.shape[1]).transpose(1, 0, 2))

    wk3 = (w_k.astype(f32) * g_kv.astype(f32)[:, None]).reshape(R, H, DN + DR)
    wv2 = (w_v.astype(f32) * g_kv.astype(f32)[:, None]).reshape(R, H, DV)

    # rope tables: packed e-view col c (of 128) has angle freqs[s, c % 32]
    ang = freqs.astype(f32)  # [S, 32]
    idx = np.tile(np.arange(32), 4)
    cos4 = np.cos(ang)[:, idx].reshape(NT, 128, 128).transpose(1, 0, 2)
    sin4 = np.sin(ang)[:, idx].reshape(NT, 128, 128).transpose(1, 0, 2)
    cosb = np.ascontiguousarray(cos4).astype(bf)
    sinb = np.ascontiguousarray(sin4).astype(bf)

    wkvh, wkvl = _f8_hilo(w_kv.astype(f32) * WS)
    in_maps = []
    for c in range(8):
        b, g = c // 4, c % 4
        hs = slice(4 * g, 4 * g + 4)
        xT = np.ascontiguousarray(x[b].astype(f32).T)  # [D, S]
        xh, xl = _f8_hilo(xT)
        wqn_c = np.ascontiguousarray(
            w_qn.reshape(D, H, DN)[:, hs].reshape(D, HL * DN)).astype(f32) * WS
        wqr_c = np.ascontiguousarray(
            w_qr.reshape(D, H, DR)[:, hs].reshape(D, HL * DR)).astype(f32) * WS
        wqnh, wqnl = _f8_hilo(wqn_c)
        wqrh, wqrl = _f8_hilo(wqr_c)
        m = {
            "xh": pack_x(xh), "xl": pack_x(xl),
            "wkvh": pack_w(wkvh), "wkvl": pack_w(wkvl),
            "wqnh": pack_w(wqnh), "wqnl": pack_w(wqnl),
            "wqrh": pack_w(wqrh), "wqrl": pack_w(wqrl),
            "wkn": pack_r(np.ascontiguousarray(
                wk3[:, hs, :DN].reshape(R, HL * DN)).astype(bf)),
            "wkr": pack_r(np.ascontiguousarray(
                wk3[:, hs, DN:].reshape(R, HL * DR)).astype(bf)),
            "wv": pack_r(np.ascontiguousarray(
                wv2[:, hs].reshape(R, HL * DV)).astype(bf)),
            "cosb": cosb, "sinb": sinb,
        }
        wo4 = w_o.reshape(H, DV, D)[hs].astype(f32) * WS
        f8t = ml_dtypes.float8_e4m3
        m["woh"] = np.empty((2, 128, 2, D), f8t)
        m["wol"] = np.empty((2, 128, 2, D), f8t)
        for p in range(2):
            wop = np.ascontiguousarray(
                wo4[2 * p:2 * p + 2].transpose(1, 0, 2))
            m["woh"][p], m["wol"][p] = _f8_hilo(wop)
        in_maps.append(m)
    return in_maps


def kernel(x, freqs, w_kv, g_kv, w_k, w_v, w_qn, w_qr, w_o):
    if "nc" not in _CACHE:
        _CACHE["nc"] = _build()
    nc = _CACHE["nc"]
    in_maps = _prep_inputs(np.asarray(x), np.asarray(freqs), np.asarray(w_kv),
                           np.asarray(g_kv), np.asarray(w_k), np.asarray(w_v),
                           np.asarray(w_qn), np.asarray(w_qr), np.asarray(w_o))
    res = run_bass_kernel_spmd(nc, in_maps, list(range(8)), trace=False)
    out = np.zeros((B, S, D), np.float32)
    for c in range(8):
        out[c // 4] += res.results[c]["out"].astype(np.float32)
    out /= WS  # out_proj weights were pre-scaled for fp8
    return out
